# revision 18
# baseline (speedup 1.0000x reference)
"""Trainium2 Bass kernel for nn_BiGlobal_MPCMFuse (8 NeuronCores, SPMD).

Math (see reference):
    pcm_s  = min over 4 direction pairs of (cen[x+d]-cen[x])*(cen[x-d]-cen[x]),
             d in {(s,0),(0,s),(s,s),(s,-s)}, circular shifts, s in {13, 17}
    pcmN   = BN(pcm_s)  (train-mode BN over (B,H,W) per channel)
    wei    = SE-attention on the (H,W)-pooled pcmN  -> per-(b,c) sigmoid weights
    out    = td_wei * pcm13N + bu_wei * pcm17N

v2 kernel strategy (2 NEFF launches + host glue):
  - Shard H across 8 cores (48 rows + 17-row halo); partitions = the 256
    (b,c) planes in 2 blocks of 128; free dims = [rows, cols] halo'd.
  - Key identity: with f_d[x] = cen[x] - cen[x+d] (ONE subtraction per
    direction), the pair product is
        (cen[x+d]-cen[x])*(cen[x-d]-cen[x]) = -f_d[x] * f_d[x-d],
    so each pair costs 1 sub (over a slightly extended region) + 1 mul of
    two shifted views of the same field, instead of 2 subs + 1 mul.  The
    sign is absorbed by tracking M = -pcm with a MAX tree; the final
    per-plane affine (BN+SE folded) flips the sign back for free.
  - Odd-element AP offsets measured to keep the DVE 2x perf mode on this
    hardware, so no parity copy of cen is needed (halves the cen load).
  - Pass A emits M13/M17 = -pcm (bf16) + per-plane sum/sumsq partials
    (ScalarE activation accum).  Host combines partials, runs BN + SE
    exactly (float64), folds everything into per-plane coefficients:
        out = A13[p]*M13 + A17[p]*M17 + D[p]
  - Pass B applies that affine (ScalarE affine + DVE scalar_tensor_tensor),
    bf16 output upcast on host.
"""

import os
import sys

import numpy as np

for _p in ("/opt/trn_rl_repo",):
    if _p not in sys.path and os.path.isdir(_p):
        sys.path.insert(0, _p)

import ml_dtypes  # noqa: E402

BF16 = ml_dtypes.bfloat16

B, C, H, W = 4, 64, 384, 384
IC = C // 2
NCORES = 8
P = B * C            # 256 planes
ROWS = H // NCORES   # 48 rows per core
RH = 17              # row halo each side (max |shift| 17)
CH = 18              # col halo each side
SR = ROWS + 2 * RH   # 82 stored rows
SW = W + 2 * CH      # 420 stored cols
NBLK = 2             # 256 planes / 128 partitions
WC = 192             # col-chunk width
NCH = W // WC        # 2 chunks
EPS = 1e-5
SCALES = (13, 17)
LOADCUT = 240        # col split point for the two cE load DMAs

_cache = {}


def _build_pass_a():
    import concourse.bacc as bacc
    import concourse.tile as tile
    from concourse import mybir

    nc = bacc.Bacc()
    bf = mybir.dt.bfloat16
    f32 = mybir.dt.float32

    # cen shard split into two DRAM-contiguous col pieces so the first-chunk
    # compute can start after piece A lands (DRAM runs >= 512B keep line rate)
    cenA = nc.declare_dram_parameter("cenA", [NBLK, 128, SR, LOADCUT], bf, isOutput=False)
    cenB = nc.declare_dram_parameter("cenB", [NBLK, 128, SR, SW - LOADCUT], bf, isOutput=False)
    m13 = nc.declare_dram_parameter("m13", [NBLK, 128, NCH, ROWS, WC], bf, isOutput=True)
    m17 = nc.declare_dram_parameter("m17", [NBLK, 128, NCH, ROWS, WC], bf, isOutput=True)
    # per (block, partition): [sum13, sq13, sum17, sq17] x NCH   (sums of M!)
    stats = nc.declare_dram_parameter("stats", [NBLK, 128, 4, NCH], f32, isOutput=True)
    m_out = {13: m13, 17: m17}

    with tile.TileContext(nc) as tc:
        with (
            tc.tile_pool(name="cen", bufs=1) as cen_pool,
            tc.tile_pool(name="fld", bufs=2) as fld,
            tc.tile_pool(name="mtb", bufs=2) as mtb,
            tc.tile_pool(name="mbuf", bufs=1) as mbuf,
            tc.tile_pool(name="accp", bufs=1) as accp,
        ):
            for blk in range(NBLK):
                cE = cen_pool.tile([128, SR, SW], bf, tag="cE", name=f"cE{blk}")
                # chunk-0 compute needs only cols < LOADCUT: load that piece
                # on both HWDGE queues (sync + scalar) in parallel, cenB after
                nc.sync.dma_start(out=cE[:, :, 0:LOADCUT], in_=cenA[blk])
                nc.sync.dma_start(out=cE[:, :, LOADCUT:SW], in_=cenB[blk])
                acc = accp.tile([128, 4, NCH], f32, tag="acc", name=f"acc{blk}")

                for ch in range(NCH):
                    c0 = CH + ch * WC
                    for si, s in enumerate(SCALES):
                        M = mbuf.tile([128, ROWS, WC], bf, tag=f"M{s}", name=f"M{s}_{blk}_{ch}")
                        # For each direction d: f_d[x] = cen[x] - cen[x+d]
                        # over x in rows [-dy, ROWS) x cols [c_lo, c_lo+fc),
                        # then the (negated) pair product f[x]*f[x-d].
                        for pi, (dy, dx) in enumerate(((s, 0), (0, s), (s, s), (s, -s))):
                            fr = ROWS + dy
                            fc = WC + abs(dx)
                            c_lo = min(0, -dx)
                            f = fld.tile([128, ROWS + s, WC + s], bf, tag="f", name=f"f{pi}")
                            nc.vector.tensor_sub(
                                f[:, 0:fr, 0:fc],
                                cE[:, RH - dy : RH - dy + fr,
                                   c0 + c_lo : c0 + c_lo + fc],
                                cE[:, RH : RH + fr,
                                   c0 + c_lo + dx : c0 + c_lo + dx + fc],
                            )
                            xa = f[:, dy : dy + ROWS, -c_lo : -c_lo + WC]
                            xb = f[:, 0:ROWS, -c_lo - dx : -c_lo - dx + WC]
                            if pi == 0:
                                nc.vector.tensor_mul(M, xa, xb)
                            else:
                                mt = mtb.tile([128, ROWS, WC], bf, tag="mt", name=f"mt{pi}")
                                nc.vector.tensor_mul(mt, xa, xb)
                                nc.vector.tensor_tensor(M, M, mt, op=mybir.AluOpType.max)
                        # per-partition sum / sumsq of this chunk (ScalarE).
                        # Copy rewrites M with itself; Square dumps into the
                        # dead mt buffer so it doesn't serialize with the
                        # DMA-out of M.
                        nc.scalar.activation(
                            M, M, mybir.ActivationFunctionType.Copy,
                            accum_out=acc[:, 2 * si, ch : ch + 1],
                        )
                        nc.scalar.activation(
                            mt, M, mybir.ActivationFunctionType.Square,
                            accum_out=acc[:, 2 * si + 1, ch : ch + 1],
                        )
                        nc.sync.dma_start(out=m_out[s][blk, :, ch], in_=M)
                nc.sync.dma_start(out=stats[blk], in_=acc)
    return nc


def _build_pass_b():
    import concourse.bacc as bacc
    import concourse.tile as tile
    from concourse import mybir

    nc = bacc.Bacc()
    bf = mybir.dt.bfloat16
    f32 = mybir.dt.float32

    m13 = nc.declare_dram_parameter("m13", [NBLK, 128, NCH, ROWS, WC], bf, isOutput=False)
    m17 = nc.declare_dram_parameter("m17", [NBLK, 128, NCH, ROWS, WC], bf, isOutput=False)
    # per plane: [A13, A17, D, pad]
    coef = nc.declare_dram_parameter("coef", [NBLK, 128, 4], f32, isOutput=False)
    out = nc.declare_dram_parameter("out", [NBLK, 128, NCH, ROWS, WC], bf, isOutput=True)

    with tile.TileContext(nc) as tc:
        with (
            tc.tile_pool(name="cf", bufs=1) as cfp,
            tc.tile_pool(name="io", bufs=2) as io,
        ):
            for blk in range(NBLK):
                cf = cfp.tile([128, 4], f32, tag="cf", name=f"cf{blk}")
                nc.sync.dma_start(out=cf, in_=coef[blk])
                for ch in range(NCH):
                    t13 = io.tile([128, ROWS, WC], bf, tag="t13", name="t13")
                    t17 = io.tile([128, ROWS, WC], bf, tag="t17", name="t17")
                    if blk == 0 and ch == 0:
                        # halve the first unit's loads so the first combine
                        # starts ~13us earlier; later units pipeline fully
                        hr = ROWS // 2
                        nc.sync.dma_start(out=t13[:, 0:hr], in_=m13[blk, :, ch, 0:hr])
                        nc.sync.dma_start(out=t17[:, 0:hr], in_=m17[blk, :, ch, 0:hr])
                        nc.sync.dma_start(out=t13[:, hr:ROWS], in_=m13[blk, :, ch, hr:ROWS])
                        nc.sync.dma_start(out=t17[:, hr:ROWS], in_=m17[blk, :, ch, hr:ROWS])
                    else:
                        nc.sync.dma_start(out=t13, in_=m13[blk, :, ch])
                        nc.sync.dma_start(out=t17, in_=m17[blk, :, ch])
                    # u = A13*m13 + D  (ScalarE free affine)
                    u = io.tile([128, ROWS, WC], bf, tag="u", name="u")
                    nc.scalar.activation(
                        u, t13, mybir.ActivationFunctionType.Identity,
                        scale=cf[:, 0:1], bias=cf[:, 2:3],
                    )
                    # o = (m17 * A17) + u   (DVE fused scalar_tensor_tensor)
                    o = io.tile([128, ROWS, WC], bf, tag="o", name="o")
                    nc.vector.scalar_tensor_tensor(
                        out=o, in0=t17, scalar=cf[:, 1:2], in1=u,
                        op0=mybir.AluOpType.mult, op1=mybir.AluOpType.add,
                    )
                    nc.sync.dma_start(out=out[blk, :, ch], in_=o)
    return nc


def _shards_from_cen(cen):
    """Build per-core bf16 halo'd shard inputs {cenA, cenB} (col-split)."""
    pl = np.ascontiguousarray(cen.reshape(P, H, W)).astype(BF16)
    colsE = (np.arange(-CH, W + CH)) % W
    shards = []
    for k in range(NCORES):
        rows = (np.arange(-RH, ROWS + RH) + k * ROWS) % H
        sub = pl[:, rows, :]                       # [P, SR, W]
        e = sub[:, :, colsE].reshape(NBLK, 128, SR, SW)
        shards.append({
            "cenA": np.ascontiguousarray(e[:, :, :, :LOADCUT]),
            "cenB": np.ascontiguousarray(e[:, :, :, LOADCUT:]),
        })
    return shards


def _host_glue(stats_list, bn1_g, bn1_b, bn2_g, bn2_b,
               td_w1, td_b1, td_g1, td_be1, td_w2, td_b2, td_g2, td_be2,
               bu_w1, bu_b1, bu_g1, bu_be1, bu_w2, bu_b2, bu_g2, bu_be2):
    """Combine per-core stats, run BN + SE exactly, return per-plane coefs.

    Device sums are over M = -pcm, so sum(pcm) = -sum(M), sumsq unchanged,
    and the output affine gets negated A coefficients.
    """
    f8 = np.float64
    # stats_list[k]: [NBLK, 128, 4, NCH] -> global [P, 4]
    tot = np.zeros((P, 4), f8)
    for st in stats_list:
        tot += st.astype(f8).sum(axis=3).reshape(P, 4)
    sum13 = -tot[:, 0].reshape(B, C)
    sq13 = tot[:, 1].reshape(B, C)
    sum17 = -tot[:, 2].reshape(B, C)
    sq17 = tot[:, 3].reshape(B, C)

    n = B * H * W

    def bn_affine(sm, sq, g, b):
        mean = sm.sum(0) / n
        var = sq.sum(0) / n - mean * mean
        a = g.astype(f8) / np.sqrt(var + EPS)
        return a, b.astype(f8) - mean * a

    a1, b1 = bn_affine(sum13, sq13, bn1_g, bn1_b)   # BN for pcm13
    a2, b2 = bn_affine(sum17, sq17, bn2_g, bn2_b)   # BN for pcm17

    # (H,W)-pooled normalized pcm per (b,c)
    p13 = a1[None, :] * (sum13 / (H * W)) + b1[None, :]
    p17 = a2[None, :] * (sum17 / (H * W)) + b2[None, :]

    def se(p, w1, bb1, g1, be1, w2, bb2, g2, be2):
        y = p @ w1.astype(f8).T + bb1.astype(f8)[None, :]
        mu, v = y.mean(0), y.var(0)
        y = (y - mu) / np.sqrt(v + EPS) * g1.astype(f8) + be1.astype(f8)
        y = np.maximum(y, 0.0)
        z = y @ w2.astype(f8).T + bb2.astype(f8)[None, :]
        mu, v = z.mean(0), z.var(0)
        z = (z - mu) / np.sqrt(v + EPS) * g2.astype(f8) + be2.astype(f8)
        return 1.0 / (1.0 + np.exp(-z))

    td_wei = se(p17, td_w1, td_b1, td_g1, td_be1, td_w2, td_b2, td_g2, td_be2)
    bu_wei = se(p13, bu_w1, bu_b1, bu_g1, bu_be1, bu_w2, bu_b2, bu_g2, bu_be2)

    # out = td_wei*(a1*pcm13+b1) + bu_wei*(a2*pcm17+b2), pcm = -M
    A13 = -td_wei * a1[None, :]
    A17 = -bu_wei * a2[None, :]
    D = td_wei * b1[None, :] + bu_wei * b2[None, :]
    coef = np.zeros((P, 4), np.float32)
    coef[:, 0] = A13.reshape(P)
    coef[:, 1] = A17.reshape(P)
    coef[:, 2] = D.reshape(P)
    return coef.reshape(NBLK, 128, 4)


def _run(nc, in_maps, trace=False):
    from concourse.bass_utils import run_bass_kernel_spmd

    return run_bass_kernel_spmd(nc, in_maps, list(range(NCORES)), trace=trace)


def kernel(cen, bn1_g, bn1_b, bn2_g, bn2_b,
           td_w1, td_b1, td_g1, td_be1, td_w2, td_b2, td_g2, td_be2,
           bu_w1, bu_b1, bu_g1, bu_be1, bu_w2, bu_b2, bu_g2, bu_be2):
    cen = np.asarray(cen, np.float32)

    if "pass_a" not in _cache:
        nca = _build_pass_a()
        nca.compile()
        _cache["pass_a"] = nca
    if "pass_b" not in _cache:
        ncb = _build_pass_b()
        ncb.compile()
        _cache["pass_b"] = ncb

    in_a = _shards_from_cen(cen)
    res_a = _run(_cache["pass_a"], in_a).results

    coef = _host_glue(
        [r["stats"] for r in res_a],
        bn1_g, bn1_b, bn2_g, bn2_b,
        td_w1, td_b1, td_g1, td_be1, td_w2, td_b2, td_g2, td_be2,
        bu_w1, bu_b1, bu_g1, bu_be1, bu_w2, bu_b2, bu_g2, bu_be2,
    )

    in_b = [
        {"m13": r["m13"], "m17": r["m17"], "coef": coef} for r in res_a
    ]
    res_b = _run(_cache["pass_b"], in_b).results

    out = np.empty((P, H, W), np.float32)
    for k in range(NCORES):
        o = np.asarray(res_b[k]["out"], dtype=np.float32)   # [NBLK,128,NCH,48,WC]
        o = o.reshape(P, NCH, ROWS, WC).transpose(0, 2, 1, 3).reshape(P, ROWS, W)
        out[:, k * ROWS : (k + 1) * ROWS, :] = o
    return out.reshape(B, C, H, W)
